# revision 9
# baseline (speedup 1.0000x reference)
"""CosmosTransformerBlock on 8 Trainium2 NeuronCores.

Strategy (tensor-parallel + sequence-parallel residual):
- Attention heads (16) sharded 2/core; FF inner dim (8192) sharded 1024/core.
- Column-parallel wq/wk/wv/ff_w1, row-parallel wo/ff_w2 -> partial outputs
  ReduceScatter'd over the sequence dim (each core owns 256 tokens of the
  residual stream h).
- LayerNorm / AdaLN modulation computed on the local 256-token slice, then
  the modulated activations are transposed to feature-major layout and
  AllGather'd so every core has the full [hid, seq] activation for its
  matmul shards.
- All matmuls run in float32r (fp32 rounded to 12-bit mantissa; full PE
  throughput at moving-dim >= 256), accumulating in fp32 PSUM.
"""

import sys

import numpy as np

try:
    import concourse.bass as bass
except ImportError:  # pragma: no cover
    sys.path.insert(0, "/opt/trn_rl_repo")
    import concourse.bass as bass

import concourse.mybir as mybir
import concourse.tile as tile
from concourse import bacc
from concourse.bass_utils import run_bass_kernel_spmd

F32 = mybir.dt.float32
F32R = mybir.dt.float32r
AF = mybir.ActivationFunctionType
ALU = mybir.AluOpType

NCORES = 8
S = 2048          # sequence length
C = 2048          # hidden dim
SL = S // NCORES  # 256 tokens per core
D = 128           # head dim
HL = 2            # local heads per core
CROSS = 1024
SC = 512          # encoder sequence length
LORA = 256
FFL = 8192 // NCORES  # 1024 ff dims per core
CT = C // 128     # 16 c tiles
EPS = 1e-6
ISQD = float(D) ** -0.5


def round_fp32r(a: np.ndarray) -> np.ndarray:
    """Round fp32 to fp32r (12-bit mantissa, RNE) — what the PE consumes."""
    b = np.ascontiguousarray(a, dtype=np.float32).view(np.uint32)
    lsb = (b >> np.uint32(12)) & np.uint32(1)
    r = (b + np.uint32(0x7FF) + lsb) & np.uint32(0xFFFFF000)
    return r.view(np.float32)


def _build(nc: bacc.Bacc):
    dram = lambda n, s, d: nc.dram_tensor(n, s, d, kind="ExternalInput").ap()

    h_in = dram("h_s", [SL, C], F32R)
    wq1 = dram("wq1", [C, HL * D], F32R)
    wk1 = dram("wk1", [C, HL * D], F32R)
    wv1 = dram("wv1", [C, HL * D], F32R)
    wo1 = dram("wo1", [HL * D, C], F32R)
    wq2 = dram("wq2", [C, HL * D], F32R)
    wk2 = dram("wk2", [CROSS, HL * D], F32R)
    wv2 = dram("wv2", [CROSS, HL * D], F32R)
    wo2 = dram("wo2", [HL * D, C], F32R)
    encT = dram("encT", [CROSS, SC], F32R)
    w1f = dram("w1f", [C, FFL], F32R)
    w2f = dram("w2f", [FFL, C], F32R)
    aw1 = dram("aw1", [3, C, LORA], F32R)
    aw2 = dram("aw2", [3, LORA, 3 * C], F32R)
    emb_t = dram("emb_t", [128, CT], F32)      # embedded_timestep, scattered
    temb_r = dram("temb_r", [1, 3 * C], F32)
    cosT = dram("cosT", [D, S], F32)
    sinT = dram("sinT", [D, S], F32)
    rotm = dram("rotm", [D, D], F32R)          # lhsT of the rotate-half perm
    eye = dram("eye", [128, 128], F32R)
    onec = dram("onec", [128, 1], F32R)

    h_out = nc.dram_tensor("h_out", [SL, C], F32R, kind="ExternalOutput").ap()

    # internal DRAM (collectives chunked 4x over the feature dim)
    rs_in = [[nc.dram_tensor(f"rs_in{L}_{cc}", [S, 512], F32).ap()
              for cc in range(4)] for L in range(3)]
    rs_out = [[nc.dram_tensor(f"rs_out{L}_{cc}", [SL, 512], F32).ap()
               for cc in range(4)] for L in range(3)]
    ag_in = [nc.dram_tensor(f"ag_in{L}", [C, SL], F32R).ap() for L in range(3)]
    ag_out = [nc.dram_tensor(f"ag_out{L}", [4 * NCORES * 512, SL], F32R,
                             addr_space="Shared").ap() for L in range(3)]

    RG = [list(range(NCORES))]

    with tile.TileContext(nc) as tc:
        ctx_pool = tc.tile_pool(name="persist", bufs=1)
        persist = ctx_pool.__enter__()

        h_sb = persist.tile([128, 2, C], F32R, tag="h")
        nc.sync.dma_start(out=h_sb, in_=h_in.rearrange("(u p) c -> p u c", p=128))

        onec_sb = persist.tile([128, 1], F32R, tag="onec")
        nc.sync.dma_start(out=onec_sb, in_=onec)
        eye_sb = persist.tile([128, 128], F32R, tag="eye")
        nc.sync.dma_start(out=eye_sb, in_=eye)
        rotm_sb = persist.tile([D, D], F32R, tag="rotm")
        nc.sync.dma_start(out=rotm_sb, in_=rotm)
        emb_sb = persist.tile([128, CT], F32, tag="emb")
        nc.sync.dma_start(out=emb_sb, in_=emb_t)
        eps_sb = persist.tile([128, 1], F32, tag="eps")
        nc.vector.memset(eps_sb, EPS)

        # modulation vectors for the current sublayer, broadcast to 128 parts
        shB = persist.tile([128, C], F32, tag="shB")
        opscB = persist.tile([128, C], F32, tag="opscB")
        gateB = persist.tile([128, C], F32, tag="gateB")

        # ---------------- AdaLN: e_L = silu(emb) @ w1_L @ w2_L + temb -------
        silu_sb = persist.tile([128, CT], F32R, tag="silu")
        nc.scalar.activation(out=silu_sb, in_=emb_sb, func=AF.Silu)

        e_dram = nc.dram_tensor("e_dram", [3, 3 * C], F32).ap()

        with tc.tile_pool(name="adaln", bufs=1) as apool, \
             tc.tile_pool(name="adaln_ps", bufs=2, space="PSUM") as apsum, \
             tc.tile_pool(name="adaln_dram", bufs=1, space="DRAM") as adram:
            temb_sb = apool.tile([1, 3 * C], F32, tag="temb")
            nc.sync.dma_start(out=temb_sb, in_=temb_r)
            for L in range(3):
                aw1_sb = apool.tile([128, CT, LORA], F32R, tag="aw1")
                nc.sync.dma_start(
                    out=aw1_sb,
                    in_=aw1[L].rearrange("(t p) m -> p t m", p=128))
                y_ps = apsum.tile([1, LORA], F32, tag="yps")
                for ct in range(CT):
                    nc.tensor.matmul(y_ps, lhsT=silu_sb[:, ct:ct + 1],
                                     rhs=aw1_sb[:, ct, :],
                                     start=(ct == 0), stop=(ct == CT - 1))
                y_sb = apool.tile([1, LORA], F32R, tag="y")
                nc.scalar.copy(out=y_sb, in_=y_ps)
                yb = adram.tile([1, LORA], F32R, tag="yb")
                nc.sync.dma_start(out=yb, in_=y_sb)
                y2 = apool.tile([128, 2], F32R, tag="y2")
                nc.sync.dma_start(
                    out=y2, in_=yb.rearrange("o (t p) -> p (o t)", p=128))

                aw2_sb = apool.tile([128, 2, 12, 512], F32R, tag="aw2")
                nc.sync.dma_start(
                    out=aw2_sb,
                    in_=aw2[L].rearrange("(t p) (ch n) -> p t ch n",
                                         p=128, n=512))
                for ch in range(12):
                    e_ps = apsum.tile([1, 512], F32, tag="e_ps")
                    nc.tensor.matmul(e_ps, lhsT=y2[:, 0:1],
                                     rhs=aw2_sb[:, 0, ch, :],
                                     start=True, stop=False)
                    nc.tensor.matmul(e_ps, lhsT=y2[:, 1:2],
                                     rhs=aw2_sb[:, 1, ch, :],
                                     start=False, stop=True)
                    e_row = apool.tile([1, 512], F32, tag="e_row")
                    nc.vector.tensor_add(
                        out=e_row,
                        in0=e_ps,
                        in1=temb_sb[:, 512 * ch:512 * (ch + 1)])
                    nc.sync.dma_start(
                        out=e_dram[L:L + 1, 512 * ch:512 * (ch + 1)],
                        in_=e_row)

        def _ebcast(L, off):
            # DRAM partition-step-0 broadcast AP of e_dram[L, off:off+C]
            return bass.AP(tensor=e_dram.tensor, offset=L * 3 * C + off,
                           ap=[[0, 128], [1, C]])

        def load_modvecs(L):
            nc.sync.dma_start(out=shB, in_=_ebcast(L, 0))
            nc.sync.dma_start(out=opscB, in_=_ebcast(L, C))
            nc.vector.tensor_scalar_add(out=opscB, in0=opscB, scalar1=1.0)
            nc.sync.dma_start(out=gateB, in_=_ebcast(L, 2 * C))

        # ---------------- LN + modulate + transpose + AllGather -------------
        def modulate(L, mpool, mpsum):
            load_modvecs(L)
            xns = []
            for u in range(2):
                x = h_sb[:, u, :]
                stats = mpool.tile([128, 4, 6], F32, tag="stats")
                xv = x.rearrange("p (g n) -> p g n", n=512)
                for g in range(4):
                    nc.vector.bn_stats(out=stats[:, g, :], in_=xv[:, g, :])
                mv = mpool.tile([128, 2], F32, tag="mv")
                nc.vector.bn_aggr(out=mv, in_=stats)
                sd = mpool.tile([128, 1], F32, tag="sd")
                nc.scalar.activation(out=sd, in_=mv[:, 1:2], func=AF.Sqrt,
                                     bias=eps_sb)
                rstd = mpool.tile([128, 1], F32, tag="rstd")
                nc.vector.reciprocal(out=rstd, in_=sd)
                t1 = mpool.tile([128, C], F32, tag="mod_t1")
                nc.vector.tensor_scalar(out=t1, in0=x, scalar1=mv[:, 0:1],
                                        scalar2=rstd, op0=ALU.subtract,
                                        op1=ALU.mult)
                t2 = mpool.tile([128, C], F32, tag="mod_t2")
                nc.vector.tensor_mul(out=t2, in0=t1, in1=opscB)
                xn = mpool.tile([128, C], F32R, tag="mod_xn", name=f"xn{u}")
                nc.vector.tensor_add(out=xn, in0=t2, in1=shB)
                xns.append(xn)
            agw = ag_in[L].rearrange("(t p) (u q) -> p t u q", p=128, q=128)
            for cc in range(4):
                for tl in range(4):
                    ct = 4 * cc + tl
                    for u in range(2):
                        tp = mpsum.tile([128, 128], F32R, tag="tpsum")
                        nc.tensor.transpose(
                            tp, xns[u][:, 128 * ct:128 * (ct + 1)], eye_sb)
                        xe = mpool.tile([128, 128], F32R, tag="mod_xe")
                        nc.scalar.copy(out=xe, in_=tp)
                        nc.sync.dma_start(out=agw[:, ct, u, :], in_=xe)
                nc.gpsimd.collective_compute(
                    "AllGather", ALU.bypass, replica_groups=RG,
                    ins=[ag_in[L][512 * cc:512 * (cc + 1), :]],
                    outs=[ag_out[L][4096 * cc:4096 * (cc + 1), :]])

        # xn reader view: [p, cchunk, tlocal, rank, s_loc]
        def agv_of(L):
            return ag_out[L].rearrange("(cc r t p) s -> p cc t r s",
                                       p=128, r=NCORES, cc=4)

        # rms(+rope) epilogue: ps [128, 512] -> dst [128, 512] f32r
        def rms_epilogue(ps, dst, ch, rope, pool, psum):
            sq = pool.tile([128, 512], F32R, tag="ep_sq")
            nc.scalar.activation(out=sq, in_=ps, func=AF.Square)
            ss = psum.tile([1, 512], F32, tag="ep_ss")
            nc.tensor.matmul(ss, lhsT=onec_sb, rhs=sq, start=True, stop=True)
            sd = pool.tile([1, 512], F32, tag="ep_sd")
            nc.scalar.activation(out=sd, in_=ss, func=AF.Sqrt,
                                 scale=1.0 / D, bias=eps_sb[0:1, :])
            rstd = pool.tile([1, 512], F32, tag="ep_rstd")
            nc.vector.reciprocal(out=rstd, in_=sd)
            rb = pool.tile([128, 512], F32, tag="ep_rb")
            nc.gpsimd.partition_broadcast(out_ap=rb, in_ap=rstd)
            if rope:
                qe = pool.tile([128, 512], F32R, tag="ep_qe")
                nc.scalar.copy(out=qe, in_=ps)
                rot = psum.tile([128, 512], F32, tag="ep_rot")
                nc.tensor.matmul(rot, lhsT=rotm_sb, rhs=qe,
                                 start=True, stop=True)
                t1 = pool.tile([128, 512], F32, tag="ep_t1")
                nc.vector.tensor_mul(out=t1, in0=qe,
                                     in1=cos_sb[:, 512 * ch:512 * (ch + 1)])
                t2 = pool.tile([128, 512], F32, tag="ep_t2")
                nc.vector.tensor_mul(out=t2, in0=rot,
                                     in1=sin_sb[:, 512 * ch:512 * (ch + 1)])
                nc.vector.tensor_add(out=t1, in0=t1, in1=t2)
                nc.vector.tensor_mul(out=dst, in0=t1, in1=rb)
            else:
                nc.vector.tensor_mul(out=dst, in0=ps, in1=rb)

        # sdpa for one (chunk, head): writes normalized A^T [d, 512] f32r
        def sdpa(q_sb, k_sb, v_sb, nkt, h, ch, dst, pool, psum):
            av = psum.tile([128, 512], F32, tag="sd_av")
            den = psum.tile([1, 512], F32, tag="sd_den")
            for kt in range(nkt):
                sps = psum.tile([128, 512], F32, tag="sd_s")
                nc.tensor.matmul(
                    sps, lhsT=k_sb[:, h, 128 * kt:128 * (kt + 1)],
                    rhs=q_sb[:, h, 512 * ch:512 * (ch + 1)],
                    start=True, stop=True)
                pt = pool.tile([128, 512], F32R, tag="sd_pt")
                nc.scalar.activation(out=pt, in_=sps, func=AF.Exp, scale=ISQD)
                nc.tensor.matmul(den, lhsT=onec_sb, rhs=pt,
                                 start=(kt == 0), stop=(kt == nkt - 1))
                nc.tensor.matmul(av, lhsT=v_sb[:, kt, 128 * h:128 * (h + 1)],
                                 rhs=pt, start=(kt == 0), stop=(kt == nkt - 1))
            rec = pool.tile([1, 512], F32, tag="sd_rec")
            nc.vector.reciprocal(out=rec, in_=den)
            rb = pool.tile([128, 512], F32, tag="sd_rb")
            nc.gpsimd.partition_broadcast(out_ap=rb, in_ap=rec)
            nc.vector.tensor_mul(out=dst, in0=av, in1=rb)

        # O = A @ wo (one chunk of q): psum [s_tile, c_chunk], evict, DMA
        def wo_phase(a0, a1, wo_sb, ch, L, pool, psum):
            for ss in range(4):
                st = 4 * ch + ss
                for cc in range(4):
                    ops = psum.tile([128, 512], F32, tag="wo_ps")
                    nc.tensor.matmul(
                        ops, lhsT=a0[:, 128 * ss:128 * (ss + 1)],
                        rhs=wo_sb[:, 0, 512 * cc:512 * (cc + 1)],
                        start=True, stop=False)
                    nc.tensor.matmul(
                        ops, lhsT=a1[:, 128 * ss:128 * (ss + 1)],
                        rhs=wo_sb[:, 1, 512 * cc:512 * (cc + 1)],
                        start=False, stop=True)
                    oe = pool.tile([128, 512], F32, tag="wo_oe")
                    nc.scalar.copy(out=oe, in_=ops)
                    rsv = rs_in[L][cc].rearrange("(st p) c -> p st c", p=128)
                    nc.sync.dma_start(out=rsv[:, st, :], in_=oe)

        def reduce_and_residual(L):
            for cc in range(4):
                nc.gpsimd.collective_compute(
                    "ReduceScatter", ALU.add, replica_groups=RG,
                    ins=[rs_in[L][cc]], outs=[rs_out[L][cc]])
            with tc.tile_pool(name=f"res{L}", bufs=2) as rpool:
                for cc in range(4):
                    rov = rs_out[L][cc].rearrange("(u p) c -> p u c", p=128)
                    for u in range(2):
                        ot = rpool.tile([128, 512], F32, tag="res_o")
                        nc.sync.dma_start(out=ot, in_=rov[:, u, :])
                        gt = rpool.tile([128, 512], F32, tag="res_g")
                        nc.vector.tensor_mul(
                            out=gt, in0=ot,
                            in1=gateB[:, 512 * cc:512 * (cc + 1)])
                        nc.vector.tensor_add(
                            out=h_sb[:, u, 512 * cc:512 * (cc + 1)],
                            in0=h_sb[:, u, 512 * cc:512 * (cc + 1)], in1=gt)

        # ======================= sublayer 0: self attention ================
        with tc.tile_pool(name="mod0", bufs=2) as mpool, \
             tc.tile_pool(name="mod0ps", bufs=2, space="PSUM") as mpsum:
            modulate(0, mpool, mpsum)

        with tc.tile_pool(name="attn1", bufs=1) as apool:
            q_sb = apool.tile([D, HL, S], F32R, tag="q")
            k_sb = apool.tile([D, HL, S], F32R, tag="k")
            v_sb = apool.tile([128, CT, HL * D], F32R, tag="v")
            agv = agv_of(0)

            with tc.tile_pool(name="qkv1", bufs=1) as wpool, \
                 tc.tile_pool(name="qkv1x", bufs=4) as xpool, \
                 tc.tile_pool(name="qkv1w", bufs=1) as epool, \
                 tc.tile_pool(name="qkv1ps", bufs=1, space="PSUM") as qpsum, \
                 tc.tile_pool(name="qkv1ps2", bufs=1, space="PSUM") as qpsum2:
                cos_sb = apool.tile([D, S], F32, tag="cos")
                nc.sync.dma_start(out=cos_sb, in_=cosT)
                sin_sb = apool.tile([D, S], F32, tag="sin")
                nc.sync.dma_start(out=sin_sb, in_=sinT)
                wq_sb = wpool.tile([128, CT, HL * D], F32R, tag="wq")
                nc.sync.dma_start(out=wq_sb,
                                  in_=wq1.rearrange("(t p) m -> p t m", p=128))
                wk_sb = wpool.tile([128, CT, HL * D], F32R, tag="wk")
                nc.sync.dma_start(out=wk_sb,
                                  in_=wk1.rearrange("(t p) m -> p t m", p=128))
                wv_sb = wpool.tile([128, CT, HL * D], F32R, tag="wv")
                nc.sync.dma_start(out=wv_sb,
                                  in_=wv1.rearrange("(t p) m -> p t m", p=128))
                vT_sb = wpool.tile([D, HL, S], F32R, tag="vT")
                for ch in range(4):
                    pq = [qpsum.tile([128, 512], F32, tag=f"pq{j}", name=f"pq{j}")
                          for j in range(2)]
                    pk = [qpsum.tile([128, 512], F32, tag=f"pk{j}", name=f"pk{j}")
                          for j in range(2)]
                    pv = [qpsum.tile([128, 512], F32, tag=f"pv{j}", name=f"pv{j}")
                          for j in range(2)]
                    for ct in range(CT):
                        st, sp = (ct == 0), (ct == CT - 1)
                        xt = xpool.tile([128, 512], F32R, tag="xt")
                        nc.sync.dma_start(out=xt,
                                          in_=agv[:, ct // 4, ct % 4, 2 * ch:2 * ch + 2, :])
                        for hh in range(HL):
                            nc.tensor.matmul(
                                pq[hh],
                                lhsT=wq_sb[:, ct, 128 * hh:128 * (hh + 1)],
                                rhs=xt, start=st, stop=sp)
                            nc.tensor.matmul(
                                pk[hh],
                                lhsT=wk_sb[:, ct, 128 * hh:128 * (hh + 1)],
                                rhs=xt, start=st, stop=sp)
                            nc.tensor.matmul(
                                pv[hh],
                                lhsT=wv_sb[:, ct, 128 * hh:128 * (hh + 1)],
                                rhs=xt, start=st, stop=sp)
                    for hh in range(HL):
                        nc.scalar.copy(
                            out=vT_sb[:, hh, 512 * ch:512 * (ch + 1)],
                            in_=pv[hh])
                        rms_epilogue(pq[hh], q_sb[:, hh, 512 * ch:512 * (ch + 1)],
                                     ch, True, epool, qpsum2)
                        rms_epilogue(pk[hh], k_sb[:, hh, 512 * ch:512 * (ch + 1)],
                                     ch, True, epool, qpsum2)
                # V^T -> V (k-major) for the AV matmuls
                for hh in range(HL):
                    for kt in range(CT):
                        vtp = qpsum2.tile([128, 128], F32R, tag="ep_rot",
                                          name="vtp")
                        nc.tensor.transpose(
                            vtp, vT_sb[:, hh, 128 * kt:128 * (kt + 1)], eye_sb)
                        nc.scalar.copy(
                            out=v_sb[:, kt, 128 * hh:128 * (hh + 1)], in_=vtp)

            with tc.tile_pool(name="sdpa1", bufs=2) as spool, \
                 tc.tile_pool(name="sdpa1ps", bufs=2, space="PSUM") as spsum:
                wo_sb = spool.tile([128, 2, C], F32R, tag="wo", bufs=1)
                nc.sync.dma_start(out=wo_sb,
                                  in_=wo1.rearrange("(t p) m -> p t m", p=128))
                for ch in range(4):
                    a0 = spool.tile([D, 512], F32R, tag="a0")
                    a1 = spool.tile([D, 512], F32R, tag="a1")
                    sdpa(q_sb, k_sb, v_sb, CT, 0, ch, a0, spool, spsum)
                    sdpa(q_sb, k_sb, v_sb, CT, 1, ch, a1, spool, spsum)
                    wo_phase(a0, a1, wo_sb, ch, 0, spool, spsum)

        reduce_and_residual(0)

        # ======================= sublayer 1: cross attention ===============
        with tc.tile_pool(name="mod1", bufs=2) as mpool, \
             tc.tile_pool(name="mod1ps", bufs=2, space="PSUM") as mpsum:
            modulate(1, mpool, mpsum)

        with tc.tile_pool(name="attn2", bufs=1) as apool:
            q2_sb = apool.tile([D, HL, S], F32R, tag="q2")
            k2_sb = apool.tile([D, HL, SC], F32R, tag="k2")
            v2_sb = apool.tile([128, 4, HL * D], F32R, tag="v2")
            agv = agv_of(1)

            with tc.tile_pool(name="kv2", bufs=1) as wpool, \
                 tc.tile_pool(name="kv2e", bufs=1) as epool, \
                 tc.tile_pool(name="kv2ps", bufs=1, space="PSUM") as qpsum, \
                 tc.tile_pool(name="kv2ps2", bufs=1, space="PSUM") as qpsum2:
                enc_sb = wpool.tile([128, 8, SC], F32R, tag="enc")
                nc.sync.dma_start(out=enc_sb,
                                  in_=encT.rearrange("(t p) s -> p t s", p=128))
                wk2_sb = wpool.tile([128, 8, HL * D], F32R, tag="wk2")
                nc.sync.dma_start(out=wk2_sb,
                                  in_=wk2.rearrange("(t p) m -> p t m", p=128))
                wv2_sb = wpool.tile([128, 8, HL * D], F32R, tag="wv2")
                nc.sync.dma_start(out=wv2_sb,
                                  in_=wv2.rearrange("(t p) m -> p t m", p=128))
                vT2_sb = wpool.tile([D, HL, SC], F32R, tag="vT2")
                for hh in range(HL):
                    pk2 = qpsum.tile([128, SC], F32, tag="pk2")
                    pv2 = qpsum.tile([128, SC], F32, tag="pv2")
                    for t in range(8):
                        nc.tensor.matmul(
                            pk2, lhsT=wk2_sb[:, t, 128 * hh:128 * (hh + 1)],
                            rhs=enc_sb[:, t, :],
                            start=(t == 0), stop=(t == 7))
                        nc.tensor.matmul(
                            pv2, lhsT=wv2_sb[:, t, 128 * hh:128 * (hh + 1)],
                            rhs=enc_sb[:, t, :],
                            start=(t == 0), stop=(t == 7))
                    nc.scalar.copy(out=vT2_sb[:, hh, :], in_=pv2)
                    rms_epilogue(pk2, k2_sb[:, hh, :], 0, False, epool, qpsum2)
                for hh in range(HL):
                    for kt in range(4):
                        vtp2 = qpsum2.tile([128, 128], F32R, tag="ep_rot",
                                           name="vtp2")
                        nc.tensor.transpose(
                            vtp2, vT2_sb[:, hh, 128 * kt:128 * (kt + 1)],
                            eye_sb)
                        nc.scalar.copy(
                            out=v2_sb[:, kt, 128 * hh:128 * (hh + 1)],
                            in_=vtp2)

                wq2_sb = wpool.tile([128, CT, HL * D], F32R, tag="wq2")
                nc.sync.dma_start(out=wq2_sb,
                                  in_=wq2.rearrange("(t p) m -> p t m", p=128))
                for ch in range(4):
                    pq = [qpsum.tile([128, 512], F32, tag=f"pq2_{j}", name=f"pq2_{j}")
                          for j in range(2)]
                    for ct in range(CT):
                        st, sp = (ct == 0), (ct == CT - 1)
                        xt = wpool.tile([128, 512], F32R, tag="xt2", bufs=4)
                        nc.sync.dma_start(out=xt,
                                          in_=agv[:, ct // 4, ct % 4, 2 * ch:2 * ch + 2, :])
                        for hh in range(HL):
                            nc.tensor.matmul(
                                pq[hh],
                                lhsT=wq2_sb[:, ct, 128 * hh:128 * (hh + 1)],
                                rhs=xt, start=st, stop=sp)
                    for hh in range(HL):
                        rms_epilogue(pq[hh],
                                     q2_sb[:, hh, 512 * ch:512 * (ch + 1)],
                                     ch, False, epool, qpsum2)

            with tc.tile_pool(name="sdpa2", bufs=2) as spool, \
                 tc.tile_pool(name="sdpa2ps", bufs=2, space="PSUM") as spsum:
                wo2_sb = spool.tile([128, 2, C], F32R, tag="wo2", bufs=1)
                nc.sync.dma_start(out=wo2_sb,
                                  in_=wo2.rearrange("(t p) m -> p t m", p=128))
                for ch in range(4):
                    a0 = spool.tile([D, 512], F32R, tag="a20")
                    a1 = spool.tile([D, 512], F32R, tag="a21")
                    sdpa(q2_sb, k2_sb, v2_sb, 4, 0, ch, a0, spool, spsum)
                    sdpa(q2_sb, k2_sb, v2_sb, 4, 1, ch, a1, spool, spsum)
                    wo_phase(a0, a1, wo2_sb, ch, 1, spool, spsum)

        reduce_and_residual(1)

        # ======================= sublayer 2: feed forward ==================
        with tc.tile_pool(name="mod2", bufs=2) as mpool, \
             tc.tile_pool(name="mod2ps", bufs=2, space="PSUM") as mpsum:
            modulate(2, mpool, mpsum)

        with tc.tile_pool(name="ff", bufs=1) as fpool:
            g_sb = fpool.tile([128, 8, S], F32R, tag="g")
            agv = agv_of(2)
            with tc.tile_pool(name="ff1", bufs=1) as wpool, \
                 tc.tile_pool(name="ff1ps", bufs=2, space="PSUM") as fpsum:
                w1_sb = wpool.tile([128, CT, FFL], F32R, tag="w1")
                nc.sync.dma_start(out=w1_sb,
                                  in_=w1f.rearrange("(t p) m -> p t m", p=128))
                for ch in range(4):
                    xncol = wpool.tile([128, CT, 512], F32R, tag="xncolf")
                    for ct in range(CT):
                        nc.sync.dma_start(out=xncol[:, ct, :],
                                          in_=agv[:, ct // 4, ct % 4, 2 * ch:2 * ch + 2, :])
                    for ft in range(8):
                        gps = fpsum.tile([128, 512], F32, tag="gps")
                        for ct in range(CT):
                            nc.tensor.matmul(
                                gps, lhsT=w1_sb[:, ct, 128 * ft:128 * (ft + 1)],
                                rhs=xncol[:, ct, :],
                                start=(ct == 0), stop=(ct == CT - 1))
                        nc.scalar.activation(
                            out=g_sb[:, ft, 512 * ch:512 * (ch + 1)],
                            in_=gps, func=AF.Gelu)  # xncol reused by 8 ft

            with tc.tile_pool(name="ff2", bufs=2) as wpool, \
                 tc.tile_pool(name="ff2ps", bufs=3, space="PSUM") as fpsum:
                w2v = w2f.rearrange("(t p) (cc n) -> p t cc n", p=128, n=512)
                for cc in range(4):
                    w2c = wpool.tile([128, 8, 512], F32R, tag="w2c")
                    nc.sync.dma_start(out=w2c, in_=w2v[:, :, cc, :])
                    for st in range(CT):
                        yps = fpsum.tile([128, 512], F32, tag="yps")
                        for ft in range(8):
                            nc.tensor.matmul(
                                yps, lhsT=g_sb[:, ft, 128 * st:128 * (st + 1)],
                                rhs=w2c[:, ft, :],
                                start=(ft == 0), stop=(ft == 7))
                        ye = wpool.tile([128, 512], F32, tag="ye")
                        nc.scalar.copy(out=ye, in_=yps)
                        rsv = rs_in[2][cc].rearrange("(st p) c -> p st c",
                                                     p=128)
                        nc.sync.dma_start(out=rsv[:, st, :], in_=ye)

        reduce_and_residual(2)

        nc.sync.dma_start(out=h_out.rearrange("(u p) c -> p u c", p=128),
                          in_=h_sb)

        ctx_pool.__exit__(None, None, None)

    nc.compile()
    return nc


_NC_CACHE = None


def _get_nc():
    global _NC_CACHE
    if _NC_CACHE is None:
        nc = bacc.Bacc("TRN2", target_bir_lowering=False, debug=False,
                       num_devices=NCORES)
        _NC_CACHE = _build(nc)
    return _NC_CACHE


def kernel(**inputs) -> np.ndarray:
    h = np.asarray(inputs["hidden_states"], np.float32)[0]      # [S, C]
    enc = np.asarray(inputs["encoder_hidden_states"], np.float32)[0]
    emb = np.asarray(inputs["embedded_timestep"], np.float32)[0]  # [C]
    temb = np.asarray(inputs["temb"], np.float32)                # [1, 3C]
    cos = np.asarray(inputs["rope_cos"], np.float32)             # [S, D]
    sin = np.asarray(inputs["rope_sin"], np.float32)

    # rms-norm affine weights are ones per the module config; verify.
    for k in ("attn1_qn", "attn1_kn", "attn2_qn", "attn2_kn"):
        assert np.allclose(np.asarray(inputs[k]), 1.0), f"{k} != ones"

    rot = np.zeros((D, D), np.float32)  # rot_out = rot_m.T @ q
    for d in range(D // 2):
        rot[64 + d, d] = -1.0   # out[d] = -q[64+d]
        rot[d, 64 + d] = 1.0    # out[64+d] = q[d]

    r = round_fp32r
    common = {
        "encT": r(enc.T), "aw1": r(np.stack([inputs["a1_w1"],
                                             inputs["a2_w1"],
                                             inputs["a3_w1"]])),
        "aw2": r(np.stack([inputs["a1_w2"], inputs["a2_w2"],
                           inputs["a3_w2"]])),
        "emb_t": np.ascontiguousarray(emb.reshape(CT, 128).T),
        "temb_r": np.ascontiguousarray(temb),
        "cosT": np.ascontiguousarray(cos.T), "sinT": np.ascontiguousarray(sin.T),
        "rotm": r(rot), "eye": r(np.eye(128, dtype=np.float32)),
        "onec": np.ones((128, 1), np.float32),
    }
    in_maps = []
    for i in range(NCORES):
        hs = slice(HL * D * i, HL * D * (i + 1))   # head-dim slice (256)
        fs = slice(FFL * i, FFL * (i + 1))         # ff slice (1024)
        m = dict(common)
        m["h_s"] = r(h[SL * i:SL * (i + 1), :])
        m["wq1"] = r(np.asarray(inputs["attn1_wq"])[:, hs])
        m["wk1"] = r(np.asarray(inputs["attn1_wk"])[:, hs])
        m["wv1"] = r(np.asarray(inputs["attn1_wv"])[:, hs])
        m["wo1"] = r(np.asarray(inputs["attn1_wo"])[hs, :])
        m["wq2"] = r(np.asarray(inputs["attn2_wq"])[:, hs])
        m["wk2"] = r(np.asarray(inputs["attn2_wk"])[:, hs])
        m["wv2"] = r(np.asarray(inputs["attn2_wv"])[:, hs])
        m["wo2"] = r(np.asarray(inputs["attn2_wo"])[hs, :])
        m["w1f"] = r(np.asarray(inputs["ff_w1"])[:, fs])
        m["w2f"] = r(np.asarray(inputs["ff_w2"])[fs, :])
        in_maps.append({k: np.ascontiguousarray(v, np.float32)
                        for k, v in m.items()})

    nc = _get_nc()
    res = run_bass_kernel_spmd(nc, in_maps, core_ids=list(range(NCORES)))
    out = np.concatenate([res.results[i]["h_out"] for i in range(NCORES)],
                         axis=0)
    return out.reshape(1, S, C).astype(np.float32)


if __name__ == "__main__":
    _get_nc()
    print("build + compile OK")


# revision 10
# speedup vs baseline: 11.5773x; 11.5773x over previous
"""CosmosTransformerBlock on 8 Trainium2 NeuronCores.

Strategy (tensor-parallel + sequence-parallel residual):
- Attention heads (16) sharded 2/core; FF inner dim (8192) sharded 1024/core.
- Column-parallel wq/wk/wv/ff_w1, row-parallel wo/ff_w2 -> partial outputs
  ReduceScatter'd over the sequence dim (each core owns 256 tokens of the
  residual stream h).
- LayerNorm / AdaLN modulation computed on the local 256-token slice, then
  the modulated activations are transposed to feature-major layout and
  AllGather'd so every core has the full [hid, seq] activation for its
  matmul shards.
- All matmuls run in float32r (fp32 rounded to 12-bit mantissa; full PE
  throughput at moving-dim >= 256), accumulating in fp32 PSUM.
"""

import sys

import numpy as np

try:
    import concourse.bass as bass
except ImportError:  # pragma: no cover
    sys.path.insert(0, "/opt/trn_rl_repo")
    import concourse.bass as bass

import concourse.mybir as mybir
import concourse.tile as tile
from concourse import bacc
from concourse.bass_utils import run_bass_kernel_spmd

F32 = mybir.dt.float32
F32R = mybir.dt.float32r
AF = mybir.ActivationFunctionType
ALU = mybir.AluOpType

NCORES = 8
S = 2048          # sequence length
C = 2048          # hidden dim
SL = S // NCORES  # 256 tokens per core
D = 128           # head dim
HL = 2            # local heads per core
CROSS = 1024
SC = 512          # encoder sequence length
LORA = 256
FFL = 8192 // NCORES  # 1024 ff dims per core
CT = C // 128     # 16 c tiles
EPS = 1e-6
ISQD = float(D) ** -0.5


def round_fp32r(a: np.ndarray) -> np.ndarray:
    """Round fp32 to fp32r (12-bit mantissa, RNE) — what the PE consumes."""
    b = np.ascontiguousarray(a, dtype=np.float32).view(np.uint32)
    lsb = (b >> np.uint32(12)) & np.uint32(1)
    r = (b + np.uint32(0x7FF) + lsb) & np.uint32(0xFFFFF000)
    return r.view(np.float32)


def _build(nc: bacc.Bacc):
    dram = lambda n, s, d: nc.dram_tensor(n, s, d, kind="ExternalInput").ap()

    h_in = dram("h_s", [SL, C], F32R)
    wq1 = dram("wq1", [C, HL * D], F32R)
    wk1 = dram("wk1", [C, HL * D], F32R)
    wv1 = dram("wv1", [C, HL * D], F32R)
    wo1 = dram("wo1", [HL * D, C], F32R)
    wq2 = dram("wq2", [C, HL * D], F32R)
    wk2 = dram("wk2", [CROSS, HL * D], F32R)
    wv2 = dram("wv2", [CROSS, HL * D], F32R)
    wo2 = dram("wo2", [HL * D, C], F32R)
    encT = dram("encT", [CROSS, SC], F32R)
    w1f = dram("w1f", [C, FFL], F32R)
    w2f = dram("w2f", [FFL, C], F32R)
    aw1 = dram("aw1", [3, C, LORA], F32R)
    aw2 = dram("aw2", [3, LORA, 3 * C], F32R)
    emb_t = dram("emb_t", [128, CT], F32)      # embedded_timestep, scattered
    temb_r = dram("temb_r", [1, 3 * C], F32)
    cosT = dram("cosT", [D, S], F32)
    sinT = dram("sinT", [D, S], F32)
    rotm = dram("rotm", [D, D], F32R)          # lhsT of the rotate-half perm
    eye = dram("eye", [128, 128], F32R)
    onec = dram("onec", [128, 1], F32R)

    h_out = nc.dram_tensor("h_out", [SL, C], F32R, kind="ExternalOutput").ap()

    # internal DRAM (collectives chunked 4x over the feature dim)
    rs_in = [[nc.dram_tensor(f"rs_in{L}_{cc}", [S, 512], F32).ap()
              for cc in range(4)] for L in range(3)]
    rs_out = [[nc.dram_tensor(f"rs_out{L}_{cc}", [SL, 512], F32).ap()
               for cc in range(4)] for L in range(3)]
    ag_in = [nc.dram_tensor(f"ag_in{L}", [C, SL], F32R).ap() for L in range(3)]
    ag_out = [nc.dram_tensor(f"ag_out{L}", [4 * NCORES * 512, SL], F32R,
                             addr_space="Shared").ap() for L in range(3)]

    RG = [list(range(NCORES))]

    with tile.TileContext(nc) as tc:
        ctx_pool = tc.tile_pool(name="persist", bufs=1)
        persist = ctx_pool.__enter__()

        h_sb = persist.tile([128, 2, C], F32R, tag="h")
        nc.sync.dma_start(out=h_sb, in_=h_in.rearrange("(u p) c -> p u c", p=128))

        onec_sb = persist.tile([128, 1], F32R, tag="onec")
        nc.sync.dma_start(out=onec_sb, in_=onec)
        eye_sb = persist.tile([128, 128], F32R, tag="eye")
        nc.sync.dma_start(out=eye_sb, in_=eye)
        rotm_sb = persist.tile([D, D], F32R, tag="rotm")
        nc.sync.dma_start(out=rotm_sb, in_=rotm)
        emb_sb = persist.tile([128, CT], F32, tag="emb")
        nc.sync.dma_start(out=emb_sb, in_=emb_t)
        eps_sb = persist.tile([128, 1], F32, tag="eps")
        nc.vector.memset(eps_sb, EPS)

        # modulation vectors for the current sublayer, broadcast to 128 parts
        shB = persist.tile([128, C], F32, tag="shB")
        opscB = persist.tile([128, C], F32, tag="opscB")
        gateB = persist.tile([128, C], F32, tag="gateB")

        # ---------------- AdaLN: e_L = silu(emb) @ w1_L @ w2_L + temb -------
        silu_sb = persist.tile([128, CT], F32R, tag="silu")
        nc.scalar.activation(out=silu_sb, in_=emb_sb, func=AF.Silu)

        e_dram = nc.dram_tensor("e_dram", [3, 3 * C], F32).ap()

        with tc.tile_pool(name="adaln", bufs=1) as apool, \
             tc.tile_pool(name="adaln_ps", bufs=2, space="PSUM") as apsum, \
             tc.tile_pool(name="adaln_dram", bufs=1, space="DRAM") as adram:
            temb_sb = apool.tile([1, 3 * C], F32, tag="temb")
            nc.sync.dma_start(out=temb_sb, in_=temb_r)
            for L in range(3):
                aw1_sb = apool.tile([128, CT, LORA], F32R, tag="aw1")
                nc.sync.dma_start(
                    out=aw1_sb,
                    in_=aw1[L].rearrange("(t p) m -> p t m", p=128))
                y_ps = apsum.tile([1, LORA], F32, tag="yps")
                for ct in range(CT):
                    nc.tensor.matmul(y_ps, lhsT=silu_sb[:, ct:ct + 1],
                                     rhs=aw1_sb[:, ct, :],
                                     start=(ct == 0), stop=(ct == CT - 1))
                y_sb = apool.tile([1, LORA], F32R, tag="y")
                nc.scalar.copy(out=y_sb, in_=y_ps)
                yb = adram.tile([1, LORA], F32R, tag="yb")
                nc.sync.dma_start(out=yb, in_=y_sb)
                y2 = apool.tile([128, 2], F32R, tag="y2")
                nc.sync.dma_start(
                    out=y2, in_=yb.rearrange("o (t p) -> p (o t)", p=128))

                aw2_sb = apool.tile([128, 2, 12, 512], F32R, tag="aw2")
                nc.sync.dma_start(
                    out=aw2_sb,
                    in_=aw2[L].rearrange("(t p) (ch n) -> p t ch n",
                                         p=128, n=512))
                for ch in range(12):
                    e_ps = apsum.tile([1, 512], F32, tag="e_ps")
                    nc.tensor.matmul(e_ps, lhsT=y2[:, 0:1],
                                     rhs=aw2_sb[:, 0, ch, :],
                                     start=True, stop=False)
                    nc.tensor.matmul(e_ps, lhsT=y2[:, 1:2],
                                     rhs=aw2_sb[:, 1, ch, :],
                                     start=False, stop=True)
                    e_row = apool.tile([1, 512], F32, tag="e_row")
                    nc.vector.tensor_add(
                        out=e_row,
                        in0=e_ps,
                        in1=temb_sb[:, 512 * ch:512 * (ch + 1)])
                    nc.sync.dma_start(
                        out=e_dram[L:L + 1, 512 * ch:512 * (ch + 1)],
                        in_=e_row)

        def _ebcast(L, off):
            # DRAM partition-step-0 broadcast AP of e_dram[L, off:off+C]
            return bass.AP(tensor=e_dram.tensor, offset=L * 3 * C + off,
                           ap=[[0, 128], [1, C]])

        def load_modvecs(L):
            nc.sync.dma_start(out=shB, in_=_ebcast(L, 0))
            nc.sync.dma_start(out=opscB, in_=_ebcast(L, C))
            nc.vector.tensor_scalar_add(out=opscB, in0=opscB, scalar1=1.0)
            nc.sync.dma_start(out=gateB, in_=_ebcast(L, 2 * C))

        # ---------------- LN + modulate + transpose + AllGather -------------
        def modulate(L, mpool, mpsum):
            load_modvecs(L)
            xns = []
            for u in range(2):
                x = h_sb[:, u, :]
                stats = mpool.tile([128, 4, 6], F32, tag="stats")
                xv = x.rearrange("p (g n) -> p g n", n=512)
                for g in range(4):
                    nc.vector.bn_stats(out=stats[:, g, :], in_=xv[:, g, :])
                mv = mpool.tile([128, 2], F32, tag="mv")
                nc.vector.bn_aggr(out=mv, in_=stats)
                sd = mpool.tile([128, 1], F32, tag="sd")
                nc.scalar.activation(out=sd, in_=mv[:, 1:2], func=AF.Sqrt,
                                     bias=eps_sb)
                rstd = mpool.tile([128, 1], F32, tag="rstd")
                nc.vector.reciprocal(out=rstd, in_=sd)
                t1 = mpool.tile([128, C], F32, tag="mod_t1")
                nc.vector.tensor_scalar(out=t1, in0=x, scalar1=mv[:, 0:1],
                                        scalar2=rstd, op0=ALU.subtract,
                                        op1=ALU.mult)
                t2 = mpool.tile([128, C], F32, tag="mod_t2")
                nc.vector.tensor_mul(out=t2, in0=t1, in1=opscB)
                xn = mpool.tile([128, C], F32R, tag="mod_xn", name=f"xn{u}")
                nc.vector.tensor_add(out=xn, in0=t2, in1=shB)
                xns.append(xn)
            agw = ag_in[L].rearrange("(t p) (u q) -> p t u q", p=128, q=128)
            for cc in range(4):
                for tl in range(4):
                    ct = 4 * cc + tl
                    for u in range(2):
                        tp = mpsum.tile([128, 128], F32R, tag="tpsum")
                        nc.tensor.transpose(
                            tp, xns[u][:, 128 * ct:128 * (ct + 1)], eye_sb)
                        xe = mpool.tile([128, 128], F32R, tag="mod_xe")
                        nc.vector.tensor_copy(out=xe, in_=tp)
                        nc.sync.dma_start(out=agw[:, ct, u, :], in_=xe)
                nc.gpsimd.collective_compute(
                    "AllGather", ALU.bypass, replica_groups=RG,
                    ins=[ag_in[L][512 * cc:512 * (cc + 1), :]],
                    outs=[ag_out[L][4096 * cc:4096 * (cc + 1), :]])

        # xn reader view: [p, cchunk, tlocal, rank, s_loc]
        def agv_of(L):
            return ag_out[L].rearrange("(cc r t p) s -> p cc t r s",
                                       p=128, r=NCORES, cc=4)

        # rms(+rope) epilogue: ps [128, 512] -> dst [128, 512] f32r
        def rms_epilogue(ps, dst, ch, rope, pool, psum):
            sq = pool.tile([128, 512], F32R, tag="ep_sq")
            nc.scalar.activation(out=sq, in_=ps, func=AF.Square)
            ss = psum.tile([1, 512], F32, tag="ep_ss")
            nc.tensor.matmul(ss, lhsT=onec_sb, rhs=sq, start=True, stop=True)
            sd = pool.tile([1, 512], F32, tag="ep_sd")
            nc.scalar.activation(out=sd, in_=ss, func=AF.Sqrt,
                                 scale=1.0 / D, bias=eps_sb[0:1, :])
            rstd = pool.tile([1, 512], F32, tag="ep_rstd")
            nc.vector.reciprocal(out=rstd, in_=sd)
            rb = pool.tile([128, 512], F32, tag="ep_rb")
            nc.gpsimd.partition_broadcast(out_ap=rb, in_ap=rstd)
            if rope:
                qe = pool.tile([128, 512], F32R, tag="ep_qe")
                nc.scalar.copy(out=qe, in_=ps)
                rot = psum.tile([128, 512], F32, tag="ep_rot")
                nc.tensor.matmul(rot, lhsT=rotm_sb, rhs=qe,
                                 start=True, stop=True)
                t1 = pool.tile([128, 512], F32, tag="ep_t1")
                nc.vector.tensor_mul(out=t1, in0=qe,
                                     in1=cos_sb[:, 512 * ch:512 * (ch + 1)])
                t2 = pool.tile([128, 512], F32, tag="ep_t2")
                nc.vector.tensor_mul(out=t2, in0=rot,
                                     in1=sin_sb[:, 512 * ch:512 * (ch + 1)])
                nc.vector.tensor_add(out=t1, in0=t1, in1=t2)
                nc.vector.tensor_mul(out=dst, in0=t1, in1=rb)
            else:
                nc.vector.tensor_mul(out=dst, in0=ps, in1=rb)

        # sdpa for one (chunk, head): writes normalized A^T [d, 512] f32r
        def sdpa(q_sb, k_sb, v_sb, nkt, h, ch, dst, pool, psum):
            av = psum.tile([128, 512], F32, tag="sd_av")
            den = psum.tile([1, 512], F32, tag="sd_den")
            for kt in range(nkt):
                sps = psum.tile([128, 512], F32, tag="sd_s")
                nc.tensor.matmul(
                    sps, lhsT=k_sb[:, h, 128 * kt:128 * (kt + 1)],
                    rhs=q_sb[:, h, 512 * ch:512 * (ch + 1)],
                    start=True, stop=True)
                pt = pool.tile([128, 512], F32R, tag="sd_pt")
                nc.scalar.activation(out=pt, in_=sps, func=AF.Exp, scale=ISQD)
                nc.tensor.matmul(den, lhsT=onec_sb, rhs=pt,
                                 start=(kt == 0), stop=(kt == nkt - 1))
                nc.tensor.matmul(av, lhsT=v_sb[:, kt, 128 * h:128 * (h + 1)],
                                 rhs=pt, start=(kt == 0), stop=(kt == nkt - 1))
            rec = pool.tile([1, 512], F32, tag="sd_rec")
            nc.vector.reciprocal(out=rec, in_=den)
            rb = pool.tile([128, 512], F32, tag="sd_rb")
            nc.gpsimd.partition_broadcast(out_ap=rb, in_ap=rec)
            nc.vector.tensor_mul(out=dst, in0=av, in1=rb)

        # O = A @ wo (one chunk of q): psum [s_tile, c_chunk], evict, DMA
        def wo_phase(a0, a1, wo_sb, ch, L, pool, psum):
            for ss in range(4):
                st = 4 * ch + ss
                for cc in range(4):
                    ops = psum.tile([128, 512], F32, tag="wo_ps")
                    nc.tensor.matmul(
                        ops, lhsT=a0[:, 128 * ss:128 * (ss + 1)],
                        rhs=wo_sb[:, 0, 512 * cc:512 * (cc + 1)],
                        start=True, stop=False)
                    nc.tensor.matmul(
                        ops, lhsT=a1[:, 128 * ss:128 * (ss + 1)],
                        rhs=wo_sb[:, 1, 512 * cc:512 * (cc + 1)],
                        start=False, stop=True)
                    oe = pool.tile([128, 512], F32, tag="wo_oe")
                    nc.vector.tensor_copy(out=oe, in_=ops)
                    rsv = rs_in[L][cc].rearrange("(st p) c -> p st c", p=128)
                    nc.sync.dma_start(out=rsv[:, st, :], in_=oe)

        def reduce_and_residual(L):
            for cc in range(4):
                nc.gpsimd.collective_compute(
                    "ReduceScatter", ALU.add, replica_groups=RG,
                    ins=[rs_in[L][cc]], outs=[rs_out[L][cc]])
            with tc.tile_pool(name=f"res{L}", bufs=2) as rpool:
                for cc in range(4):
                    rov = rs_out[L][cc].rearrange("(u p) c -> p u c", p=128)
                    for u in range(2):
                        ot = rpool.tile([128, 512], F32, tag="res_o")
                        nc.sync.dma_start(out=ot, in_=rov[:, u, :])
                        gt = rpool.tile([128, 512], F32, tag="res_g")
                        nc.vector.tensor_mul(
                            out=gt, in0=ot,
                            in1=gateB[:, 512 * cc:512 * (cc + 1)])
                        nc.vector.tensor_add(
                            out=h_sb[:, u, 512 * cc:512 * (cc + 1)],
                            in0=h_sb[:, u, 512 * cc:512 * (cc + 1)], in1=gt)

        # ======================= sublayer 0: self attention ================
        with tc.tile_pool(name="mod0", bufs=2) as mpool, \
             tc.tile_pool(name="mod0ps", bufs=2, space="PSUM") as mpsum:
            modulate(0, mpool, mpsum)

        with tc.tile_pool(name="attn1", bufs=1) as apool:
            q_sb = apool.tile([D, HL, S], F32R, tag="q")
            k_sb = apool.tile([D, HL, S], F32R, tag="k")
            v_sb = apool.tile([128, CT, HL * D], F32R, tag="v")
            agv = agv_of(0)

            with tc.tile_pool(name="qkv1", bufs=1) as wpool, \
                 tc.tile_pool(name="qkv1x", bufs=4) as xpool, \
                 tc.tile_pool(name="qkv1w", bufs=1) as epool, \
                 tc.tile_pool(name="qkv1ps", bufs=1, space="PSUM") as qpsum, \
                 tc.tile_pool(name="qkv1ps2", bufs=1, space="PSUM") as qpsum2:
                cos_sb = apool.tile([D, S], F32, tag="cos")
                nc.sync.dma_start(out=cos_sb, in_=cosT)
                sin_sb = apool.tile([D, S], F32, tag="sin")
                nc.sync.dma_start(out=sin_sb, in_=sinT)
                wq_sb = wpool.tile([128, CT, HL * D], F32R, tag="wq")
                nc.sync.dma_start(out=wq_sb,
                                  in_=wq1.rearrange("(t p) m -> p t m", p=128))
                wk_sb = wpool.tile([128, CT, HL * D], F32R, tag="wk")
                nc.sync.dma_start(out=wk_sb,
                                  in_=wk1.rearrange("(t p) m -> p t m", p=128))
                wv_sb = wpool.tile([128, CT, HL * D], F32R, tag="wv")
                nc.sync.dma_start(out=wv_sb,
                                  in_=wv1.rearrange("(t p) m -> p t m", p=128))
                vT_sb = wpool.tile([D, HL, S], F32R, tag="vT")
                for ch in range(4):
                    pq = [qpsum.tile([128, 512], F32, tag=f"pq{j}", name=f"pq{j}")
                          for j in range(2)]
                    pk = [qpsum.tile([128, 512], F32, tag=f"pk{j}", name=f"pk{j}")
                          for j in range(2)]
                    pv = [qpsum.tile([128, 512], F32, tag=f"pv{j}", name=f"pv{j}")
                          for j in range(2)]
                    for ct in range(CT):
                        st, sp = (ct == 0), (ct == CT - 1)
                        xt = xpool.tile([128, 512], F32R, tag="xt")
                        nc.sync.dma_start(out=xt,
                                          in_=agv[:, ct // 4, ct % 4, 2 * ch:2 * ch + 2, :])
                        for hh in range(HL):
                            nc.tensor.matmul(
                                pq[hh],
                                lhsT=wq_sb[:, ct, 128 * hh:128 * (hh + 1)],
                                rhs=xt, start=st, stop=sp)
                            nc.tensor.matmul(
                                pk[hh],
                                lhsT=wk_sb[:, ct, 128 * hh:128 * (hh + 1)],
                                rhs=xt, start=st, stop=sp)
                            nc.tensor.matmul(
                                pv[hh],
                                lhsT=wv_sb[:, ct, 128 * hh:128 * (hh + 1)],
                                rhs=xt, start=st, stop=sp)
                    for hh in range(HL):
                        nc.vector.tensor_copy(
                            out=vT_sb[:, hh, 512 * ch:512 * (ch + 1)],
                            in_=pv[hh])
                        rms_epilogue(pq[hh], q_sb[:, hh, 512 * ch:512 * (ch + 1)],
                                     ch, True, epool, qpsum2)
                        rms_epilogue(pk[hh], k_sb[:, hh, 512 * ch:512 * (ch + 1)],
                                     ch, True, epool, qpsum2)
                # V^T -> V (k-major) for the AV matmuls
                for hh in range(HL):
                    for kt in range(CT):
                        vtp = qpsum2.tile([128, 128], F32R, tag="ep_rot",
                                          name="vtp")
                        nc.tensor.transpose(
                            vtp, vT_sb[:, hh, 128 * kt:128 * (kt + 1)], eye_sb)
                        nc.vector.tensor_copy(
                            out=v_sb[:, kt, 128 * hh:128 * (hh + 1)], in_=vtp)

            with tc.tile_pool(name="sdpa1", bufs=2) as spool, \
                 tc.tile_pool(name="sdpa1ps", bufs=2, space="PSUM") as spsum:
                wo_sb = spool.tile([128, 2, C], F32R, tag="wo", bufs=1)
                nc.sync.dma_start(out=wo_sb,
                                  in_=wo1.rearrange("(t p) m -> p t m", p=128))
                for ch in range(4):
                    a0 = spool.tile([D, 512], F32R, tag="a0")
                    a1 = spool.tile([D, 512], F32R, tag="a1")
                    sdpa(q_sb, k_sb, v_sb, CT, 0, ch, a0, spool, spsum)
                    sdpa(q_sb, k_sb, v_sb, CT, 1, ch, a1, spool, spsum)
                    wo_phase(a0, a1, wo_sb, ch, 0, spool, spsum)

        reduce_and_residual(0)

        # ======================= sublayer 1: cross attention ===============
        with tc.tile_pool(name="mod1", bufs=2) as mpool, \
             tc.tile_pool(name="mod1ps", bufs=2, space="PSUM") as mpsum:
            modulate(1, mpool, mpsum)

        with tc.tile_pool(name="attn2", bufs=1) as apool:
            q2_sb = apool.tile([D, HL, S], F32R, tag="q2")
            k2_sb = apool.tile([D, HL, SC], F32R, tag="k2")
            v2_sb = apool.tile([128, 4, HL * D], F32R, tag="v2")
            agv = agv_of(1)

            with tc.tile_pool(name="kv2", bufs=1) as wpool, \
                 tc.tile_pool(name="kv2e", bufs=1) as epool, \
                 tc.tile_pool(name="kv2ps", bufs=1, space="PSUM") as qpsum, \
                 tc.tile_pool(name="kv2ps2", bufs=1, space="PSUM") as qpsum2:
                enc_sb = wpool.tile([128, 8, SC], F32R, tag="enc")
                nc.sync.dma_start(out=enc_sb,
                                  in_=encT.rearrange("(t p) s -> p t s", p=128))
                wk2_sb = wpool.tile([128, 8, HL * D], F32R, tag="wk2")
                nc.sync.dma_start(out=wk2_sb,
                                  in_=wk2.rearrange("(t p) m -> p t m", p=128))
                wv2_sb = wpool.tile([128, 8, HL * D], F32R, tag="wv2")
                nc.sync.dma_start(out=wv2_sb,
                                  in_=wv2.rearrange("(t p) m -> p t m", p=128))
                vT2_sb = wpool.tile([D, HL, SC], F32R, tag="vT2")
                for hh in range(HL):
                    pk2 = qpsum.tile([128, SC], F32, tag="pk2")
                    pv2 = qpsum.tile([128, SC], F32, tag="pv2")
                    for t in range(8):
                        nc.tensor.matmul(
                            pk2, lhsT=wk2_sb[:, t, 128 * hh:128 * (hh + 1)],
                            rhs=enc_sb[:, t, :],
                            start=(t == 0), stop=(t == 7))
                        nc.tensor.matmul(
                            pv2, lhsT=wv2_sb[:, t, 128 * hh:128 * (hh + 1)],
                            rhs=enc_sb[:, t, :],
                            start=(t == 0), stop=(t == 7))
                    nc.vector.tensor_copy(out=vT2_sb[:, hh, :], in_=pv2)
                    rms_epilogue(pk2, k2_sb[:, hh, :], 0, False, epool, qpsum2)
                for hh in range(HL):
                    for kt in range(4):
                        vtp2 = qpsum2.tile([128, 128], F32R, tag="ep_rot",
                                           name="vtp2")
                        nc.tensor.transpose(
                            vtp2, vT2_sb[:, hh, 128 * kt:128 * (kt + 1)],
                            eye_sb)
                        nc.vector.tensor_copy(
                            out=v2_sb[:, kt, 128 * hh:128 * (hh + 1)],
                            in_=vtp2)

                wq2_sb = wpool.tile([128, CT, HL * D], F32R, tag="wq2")
                nc.sync.dma_start(out=wq2_sb,
                                  in_=wq2.rearrange("(t p) m -> p t m", p=128))
                for ch in range(4):
                    pq = [qpsum.tile([128, 512], F32, tag=f"pq2_{j}", name=f"pq2_{j}")
                          for j in range(2)]
                    for ct in range(CT):
                        st, sp = (ct == 0), (ct == CT - 1)
                        xt = wpool.tile([128, 512], F32R, tag="xt2", bufs=4)
                        nc.sync.dma_start(out=xt,
                                          in_=agv[:, ct // 4, ct % 4, 2 * ch:2 * ch + 2, :])
                        for hh in range(HL):
                            nc.tensor.matmul(
                                pq[hh],
                                lhsT=wq2_sb[:, ct, 128 * hh:128 * (hh + 1)],
                                rhs=xt, start=st, stop=sp)
                    for hh in range(HL):
                        rms_epilogue(pq[hh],
                                     q2_sb[:, hh, 512 * ch:512 * (ch + 1)],
                                     ch, False, epool, qpsum2)

            with tc.tile_pool(name="sdpa2", bufs=2) as spool, \
                 tc.tile_pool(name="sdpa2ps", bufs=2, space="PSUM") as spsum:
                wo2_sb = spool.tile([128, 2, C], F32R, tag="wo2", bufs=1)
                nc.sync.dma_start(out=wo2_sb,
                                  in_=wo2.rearrange("(t p) m -> p t m", p=128))
                for ch in range(4):
                    a0 = spool.tile([D, 512], F32R, tag="a20")
                    a1 = spool.tile([D, 512], F32R, tag="a21")
                    sdpa(q2_sb, k2_sb, v2_sb, 4, 0, ch, a0, spool, spsum)
                    sdpa(q2_sb, k2_sb, v2_sb, 4, 1, ch, a1, spool, spsum)
                    wo_phase(a0, a1, wo2_sb, ch, 1, spool, spsum)

        reduce_and_residual(1)

        # ======================= sublayer 2: feed forward ==================
        with tc.tile_pool(name="mod2", bufs=2) as mpool, \
             tc.tile_pool(name="mod2ps", bufs=2, space="PSUM") as mpsum:
            modulate(2, mpool, mpsum)

        with tc.tile_pool(name="ff", bufs=1) as fpool:
            g_sb = fpool.tile([128, 8, S], F32R, tag="g")
            agv = agv_of(2)
            with tc.tile_pool(name="ff1", bufs=1) as wpool, \
                 tc.tile_pool(name="ff1ps", bufs=2, space="PSUM") as fpsum:
                w1_sb = wpool.tile([128, CT, FFL], F32R, tag="w1")
                nc.sync.dma_start(out=w1_sb,
                                  in_=w1f.rearrange("(t p) m -> p t m", p=128))
                for ch in range(4):
                    xncol = wpool.tile([128, CT, 512], F32R, tag="xncolf")
                    for ct in range(CT):
                        nc.sync.dma_start(out=xncol[:, ct, :],
                                          in_=agv[:, ct // 4, ct % 4, 2 * ch:2 * ch + 2, :])
                    for ft in range(8):
                        gps = fpsum.tile([128, 512], F32, tag="gps")
                        for ct in range(CT):
                            nc.tensor.matmul(
                                gps, lhsT=w1_sb[:, ct, 128 * ft:128 * (ft + 1)],
                                rhs=xncol[:, ct, :],
                                start=(ct == 0), stop=(ct == CT - 1))
                        nc.scalar.activation(
                            out=g_sb[:, ft, 512 * ch:512 * (ch + 1)],
                            in_=gps, func=AF.Gelu)  # xncol reused by 8 ft

            with tc.tile_pool(name="ff2", bufs=2) as wpool, \
                 tc.tile_pool(name="ff2ps", bufs=3, space="PSUM") as fpsum:
                w2v = w2f.rearrange("(t p) (cc n) -> p t cc n", p=128, n=512)
                for cc in range(4):
                    w2c = wpool.tile([128, 8, 512], F32R, tag="w2c")
                    nc.sync.dma_start(out=w2c, in_=w2v[:, :, cc, :])
                    for st in range(CT):
                        yps = fpsum.tile([128, 512], F32, tag="yps")
                        for ft in range(8):
                            nc.tensor.matmul(
                                yps, lhsT=g_sb[:, ft, 128 * st:128 * (st + 1)],
                                rhs=w2c[:, ft, :],
                                start=(ft == 0), stop=(ft == 7))
                        ye = wpool.tile([128, 512], F32, tag="ye")
                        nc.vector.tensor_copy(out=ye, in_=yps)
                        rsv = rs_in[2][cc].rearrange("(st p) c -> p st c",
                                                     p=128)
                        nc.sync.dma_start(out=rsv[:, st, :], in_=ye)

        reduce_and_residual(2)

        nc.sync.dma_start(out=h_out.rearrange("(u p) c -> p u c", p=128),
                          in_=h_sb)

        ctx_pool.__exit__(None, None, None)

    nc.compile()
    return nc


_NC_CACHE = None


def _get_nc():
    global _NC_CACHE
    if _NC_CACHE is None:
        nc = bacc.Bacc("TRN2", target_bir_lowering=False, debug=False,
                       num_devices=NCORES)
        _NC_CACHE = _build(nc)
    return _NC_CACHE


def kernel(**inputs) -> np.ndarray:
    h = np.asarray(inputs["hidden_states"], np.float32)[0]      # [S, C]
    enc = np.asarray(inputs["encoder_hidden_states"], np.float32)[0]
    emb = np.asarray(inputs["embedded_timestep"], np.float32)[0]  # [C]
    temb = np.asarray(inputs["temb"], np.float32)                # [1, 3C]
    cos = np.asarray(inputs["rope_cos"], np.float32)             # [S, D]
    sin = np.asarray(inputs["rope_sin"], np.float32)

    # rms-norm affine weights are ones per the module config; verify.
    for k in ("attn1_qn", "attn1_kn", "attn2_qn", "attn2_kn"):
        assert np.allclose(np.asarray(inputs[k]), 1.0), f"{k} != ones"

    rot = np.zeros((D, D), np.float32)  # rot_out = rot_m.T @ q
    for d in range(D // 2):
        rot[64 + d, d] = -1.0   # out[d] = -q[64+d]
        rot[d, 64 + d] = 1.0    # out[64+d] = q[d]

    r = round_fp32r
    common = {
        "encT": r(enc.T), "aw1": r(np.stack([inputs["a1_w1"],
                                             inputs["a2_w1"],
                                             inputs["a3_w1"]])),
        "aw2": r(np.stack([inputs["a1_w2"], inputs["a2_w2"],
                           inputs["a3_w2"]])),
        "emb_t": np.ascontiguousarray(emb.reshape(CT, 128).T),
        "temb_r": np.ascontiguousarray(temb),
        "cosT": np.ascontiguousarray(cos.T), "sinT": np.ascontiguousarray(sin.T),
        "rotm": r(rot), "eye": r(np.eye(128, dtype=np.float32)),
        "onec": np.ones((128, 1), np.float32),
    }
    in_maps = []
    for i in range(NCORES):
        hs = slice(HL * D * i, HL * D * (i + 1))   # head-dim slice (256)
        fs = slice(FFL * i, FFL * (i + 1))         # ff slice (1024)
        m = dict(common)
        m["h_s"] = r(h[SL * i:SL * (i + 1), :])
        m["wq1"] = r(np.asarray(inputs["attn1_wq"])[:, hs])
        m["wk1"] = r(np.asarray(inputs["attn1_wk"])[:, hs])
        m["wv1"] = r(np.asarray(inputs["attn1_wv"])[:, hs])
        m["wo1"] = r(np.asarray(inputs["attn1_wo"])[hs, :])
        m["wq2"] = r(np.asarray(inputs["attn2_wq"])[:, hs])
        m["wk2"] = r(np.asarray(inputs["attn2_wk"])[:, hs])
        m["wv2"] = r(np.asarray(inputs["attn2_wv"])[:, hs])
        m["wo2"] = r(np.asarray(inputs["attn2_wo"])[hs, :])
        m["w1f"] = r(np.asarray(inputs["ff_w1"])[:, fs])
        m["w2f"] = r(np.asarray(inputs["ff_w2"])[fs, :])
        in_maps.append({k: np.ascontiguousarray(v, np.float32)
                        for k, v in m.items()})

    nc = _get_nc()
    res = run_bass_kernel_spmd(nc, in_maps, core_ids=list(range(NCORES)))
    out = np.concatenate([res.results[i]["h_out"] for i in range(NCORES)],
                         axis=0)
    return out.reshape(1, S, C).astype(np.float32)


if __name__ == "__main__":
    _get_nc()
    print("build + compile OK")
